# revision 7
# baseline (speedup 1.0000x reference)
import sys

sys.path.insert(0, "/opt/trn_rl_repo")

import numpy as np

B, H, D, S, ST = 128, 32, 128, 64, 32
NCORES = 8
HPC = H // NCORES          # heads per core = 4
PASS_H = 2                 # heads resident per pass
SCALE = 0.125

# cache dtypes: K cache fp32 (64-col stationaries get no FWL anyway),
# V cache + probabilities bf16 (128-col stationaries -> FWL 2x)
KDT_NP = np.float32
import ml_dtypes
VDT_NP = ml_dtypes.bfloat16

_compiled = {}


def _patch_tile_drain(tile_mod, ScopedClock, VectorClock):
    # This container's walrus rejects multi-wait Drain instructions
    # ("Too many sync wait commands"); split the kernel-tail drain's waits
    # into one sync-engine NOP per processor.
    def _patched(self, tick_clock, wait_clock):
        vc = tick_clock.global_clock
        for proc in range(len(vc)):
            tick = vc[proc]
            if tick <= 0:
                continue
            sub = VectorClock([tick if i == proc else 0 for i in range(len(vc))])
            nop_inst = self.nc.sync.nop()
            wait_clock.add_sem_waits(nop_inst.ins, ScopedClock({None: sub}))
        self.nc.sync.drain()
        self.nc.all_engine_barrier()
        assert self.sems is not None
        popped = self.nc._tile_sem_poison_stack.pop()
        assert popped is self._sem_poison
        self.nc.clear_and_free_semaphores(list(self.sems.allocated().values()))
        self.nc.all_engine_barrier()

    tile_mod.TileContext._drain_and_barrier = _patched


def _split_multiwaits(nc, mybir):
    # this container's walrus accepts only one sem-wait per instruction:
    # hoist extra waits onto same-engine NOPs placed just before.
    n_id = 0
    for fn in nc.m.functions:
        for bb in fn.blocks:
            out = []
            for inst in bb.instructions:
                si = inst.sync_info
                if si is not None and si.on_wait and len(si.on_wait) > 1:
                    waits = list(si.on_wait)
                    for w in waits[:-1]:
                        nop = mybir.InstNoOp(name=f"mw_nop_{n_id}")
                        n_id += 1
                        nop.engine = inst.engine
                        nop.sync_info = mybir.SyncInfo(on_wait=[w], on_update=[])
                        out.append(nop)
                    inst.sync_info = mybir.SyncInfo(
                        on_wait=[waits[-1]], on_update=list(si.on_update or [])
                    )
                out.append(inst)
            bb.instructions = out


def _build(n_steps=ST, n_heads=HPC):
    import concourse.bass as bass
    import concourse.mybir as mybir
    from concourse import tile
    from concourse.vector_clock import ScopedClock, VectorClock

    _patch_tile_drain(tile, ScopedClock, VectorClock)

    f32 = mybir.dt.float32
    bf16 = mybir.dt.bfloat16
    KDT = f32 if KDT_NP is np.float32 else bf16
    VDT = bf16 if VDT_NP is ml_dtypes.bfloat16 else f32

    nc = bass.Bass()
    kT_d = nc.dram_tensor("kT", [n_heads, D, B * S], KDT, kind="ExternalInput")
    vS_d = nc.dram_tensor("vS", [n_heads, S, B * D], VDT, kind="ExternalInput")
    xT_d = nc.dram_tensor("xT", [n_heads, D, B], f32, kind="ExternalInput")
    W0_d = nc.dram_tensor("W0", [n_heads, D, 3 * D], f32, kind="ExternalInput")
    WF_d = nc.dram_tensor("WF", [n_heads, D, 3 * D], f32, kind="ExternalInput")
    Wo_d = nc.dram_tensor("Wo", [n_heads, D, D], f32, kind="ExternalInput")

    kgen_d = nc.dram_tensor("kgen", [n_heads, n_steps, D, B], f32, kind="ExternalOutput")
    vgen_d = nc.dram_tensor("vgen", [n_heads, n_steps, B, D], f32, kind="ExternalOutput")
    xout_d = nc.dram_tensor("xout", [n_heads, D, B], f32, kind="ExternalOutput")

    with tile.TileContext(nc) as tc:
        with (
            tc.tile_pool(name="cache", bufs=1) as cache_pool,
            tc.tile_pool(name="wpool", bufs=1) as wpool,
            tc.tile_pool(name="stage", bufs=4) as stage,
            tc.tile_pool(name="ppj", bufs=2, space="PSUM") as ppj,
            tc.tile_pool(name="pst", bufs=2, space="PSUM") as pst,
            tc.tile_pool(name="pzz", bufs=2, space="PSUM") as pzz,
            tc.tile_pool(name="pat", bufs=2, space="PSUM") as pat,
        ):
            ones_t = wpool.tile([S, 1], VDT)
            nc.gpsimd.memset(ones_t[:], 1.0)
            ones_1 = wpool.tile([1, D], f32)
            nc.gpsimd.memset(ones_1[:], 1.0)

            n_pass = (n_heads + PASS_H - 1) // PASS_H
            for p in range(n_pass):
                hs = [p * PASS_H + j for j in range(PASS_H) if p * PASS_H + j < n_heads]
                KT = {}
                VS = {}
                W0s = {}
                WFs = {}
                Wos = {}
                xTs = {}
                prev_aT = {}
                for h in hs:
                    KT[h] = cache_pool.tile([D, B * S], KDT, tag=f"KT{h % PASS_H}", name=f"KT{h}")
                    nc.sync.dma_start(KT[h][:], kT_d[h])
                    VS[h] = cache_pool.tile([S, B * D], VDT, tag=f"VS{h % PASS_H}", name=f"VS{h}")
                    nc.sync.dma_start(VS[h][:], vS_d[h])
                    W0s[h] = wpool.tile([D, 3 * D], f32, tag=f"W0{h % PASS_H}", name=f"W0s{h}")
                    nc.sync.dma_start(W0s[h][:], W0_d[h])
                    WFs[h] = wpool.tile([D, 3 * D], f32, tag=f"WF{h % PASS_H}", name=f"WFs{h}")
                    nc.sync.dma_start(WFs[h][:], WF_d[h])
                    Wos[h] = wpool.tile([D, D], f32, tag=f"Wo{h % PASS_H}", name=f"Wos{h}")
                    nc.sync.dma_start(Wos[h][:], Wo_d[h])
                    xTs[h] = wpool.tile([D, B], f32, tag=f"xT{h % PASS_H}", name=f"xTs{h}")
                    nc.sync.dma_start(xTs[h][:], xT_d[h])
                    prev_aT[h] = None

                for i in range(n_steps):
                    g = S - n_steps + i  # write position = 32 + i
                    for h in hs:
                        Wt = W0s[h][:] if i == 0 else WFs[h][:]
                        amov = xTs[h][:] if i == 0 else prev_aT[h][:]

                        proj = ppj.tile([D, 3 * D], f32, tag="proj")
                        # qT[e,b], kT[e,b] = W^T a ; v[b,e] = a^T W
                        nc.tensor.matmul(proj[:, 0:D], Wt[:, 0:D], amov, start=True, stop=True)
                        nc.tensor.matmul(proj[:, D:2 * D], Wt[:, D:2 * D], amov, start=True, stop=True)
                        nc.tensor.matmul(proj[:, 2 * D:3 * D], amov, Wt[:, 2 * D:3 * D], start=True, stop=True)

                        qT = stage.tile([D, B], KDT, tag="qT")
                        nc.vector.tensor_copy(qT[:], proj[:, 0:D])

                        # K cache col write: KT[d, b*64+g]
                        KT_r = KT[h][:].rearrange("d (b t) -> d b t", t=S)
                        nc.vector.tensor_copy(KT_r[:, :, g], proj[:, D:2 * D])
                        kg = stage.tile([D, B], f32, tag="kg")
                        nc.scalar.copy(kg[:], proj[:, D:2 * D])
                        nc.sync.dma_start(kgen_d[h, i], kg[:])

                        vsb = stage.tile([B, D], VDT, tag="vsb")
                        nc.vector.tensor_copy(vsb[:], proj[:, 2 * D:3 * D])
                        vg = stage.tile([B, D], f32, tag="vg")
                        nc.scalar.copy(vg[:], proj[:, 2 * D:3 * D])
                        nc.sync.dma_start(vgen_d[h, i], vg[:])
                        # V cache row write (SBUF->SBUF)
                        nc.sync.dma_start(VS[h][g:g + 1, :], vsb[:])

                        # QK: per-b matmuls assembling S^T[t, b] in psum
                        sT = pst.tile([S, B], f32, tag="sT")
                        for b in range(B):
                            nc.tensor.matmul(
                                sT[:, b:b + 1],
                                KT_r[:, b, :],
                                qT[:, b:b + 1],
                                start=True, stop=True,
                            )
                        expS = stage.tile([S, B], VDT, tag="expS")
                        nc.scalar.activation(
                            expS[:], sT[:], mybir.ActivationFunctionType.Exp,
                            scale=SCALE,
                        )
                        # Z[b] = sum_t expS ; zinv ; broadcast to [D, B]
                        zz = pzz.tile([D, 2 * D], f32, tag="zz")
                        nc.tensor.matmul(zz[0:1, D:D + B], ones_t[:], expS[:],
                                         start=True, stop=True)
                        zinv = stage.tile([1, B], f32, tag="zinv")
                        nc.vector.reciprocal(zinv[:], zz[0:1, D:D + B])
                        nc.tensor.matmul(zz[:, 0:D], ones_1[:], zinv[:],
                                         start=True, stop=True)

                        # PV: per-b matmuls assembling A^T[e, b] (unnormalized)
                        VS_r = VS[h][:].rearrange("t (b e) -> t b e", e=D)
                        aT = pat.tile([D, B], f32, tag="aT")
                        for b in range(B):
                            nc.tensor.matmul(
                                aT[:, b:b + 1],
                                VS_r[:, b, :],
                                expS[:, b:b + 1],
                                start=True, stop=True,
                            )
                        zbs = stage.tile([D, B], f32, tag="zbs")
                        nc.scalar.copy(zbs[:], zz[:, 0:D])
                        aTs = stage.tile([D, B], f32, tag="aTs")
                        nc.vector.tensor_mul(aTs[:], aT[:], zbs[:])
                        prev_aT[h] = aTs

                        if i == n_steps - 1:
                            xo_ps = ppj.tile([D, D], f32, tag="proj")
                            nc.tensor.matmul(xo_ps[:], Wos[h][:], aTs[:],
                                             start=True, stop=True)
                            xo = stage.tile([D, B], f32, tag="xo_s")
                            nc.vector.tensor_copy(xo[:], xo_ps[:])
                            nc.sync.dma_start(xout_d[h], xo[:])
    _split_multiwaits(nc, mybir)
    return nc


def _prep_core(c, x, k, v, Wq, Wk, Wv, Wo, n_heads=HPC):
    h0 = c * HPC
    hsl = slice(h0, h0 + n_heads)
    # kT[h, d, b*64+t] = k[b, h0+h, t, d]
    kT = np.ascontiguousarray(
        k[:, hsl].transpose(1, 3, 0, 2).reshape(n_heads, D, B * S)
    ).astype(KDT_NP)
    # vS[h, t, b*128+e] = v[b, h0+h, t, e]
    vS = np.ascontiguousarray(
        v[:, hsl].transpose(1, 2, 0, 3).reshape(n_heads, S, B * D)
    ).astype(VDT_NP)
    xT = np.ascontiguousarray(x[:, hsl, 0].transpose(1, 2, 0))  # [h, d, b]
    W0 = np.ascontiguousarray(
        np.stack([Wq[hsl], Wk[hsl], Wv[hsl]], axis=1).transpose(0, 2, 1, 3)
        .reshape(n_heads, D, 3 * D))
    WF = np.stack([
        np.einsum("hde,hef->hdf", Wo[hsl], Wq[hsl]),
        np.einsum("hde,hef->hdf", Wo[hsl], Wk[hsl]),
        np.einsum("hde,hef->hdf", Wo[hsl], Wv[hsl]),
    ], axis=1).transpose(0, 2, 1, 3).reshape(n_heads, D, 3 * D)
    return {
        "kT": kT,
        "vS": vS,
        "xT": xT.astype(np.float32),
        "W0": W0.astype(np.float32),
        "WF": np.ascontiguousarray(WF).astype(np.float32),
        "Wo": np.ascontiguousarray(Wo[hsl]).astype(np.float32),
    }


def kernel(x, k, v, Wq, Wk, Wv, Wo, _profile=False):
    from concourse.bass_utils import run_bass_kernel_spmd

    key = (ST, HPC)
    if key not in _compiled:
        _compiled[key] = _build(*key)
    nc = _compiled[key]

    in_maps = [_prep_core(c, x, k, v, Wq, Wk, Wv, Wo) for c in range(NCORES)]
    res = run_bass_kernel_spmd(nc, in_maps, core_ids=list(range(NCORES)),
                               trace=_profile)

    k_out = np.array(k, dtype=np.float32, copy=True)
    v_out = np.array(v, dtype=np.float32, copy=True)
    x_out = np.empty_like(np.asarray(x, dtype=np.float32))
    for c in range(NCORES):
        r = res.results[c]
        h0 = c * HPC
        # kgen [h, i, d, b] -> k_out[b, h0+h, 32+i, d]
        k_out[:, h0:h0 + HPC, S - ST:, :] = r["kgen"].transpose(3, 0, 1, 2)
        v_out[:, h0:h0 + HPC, S - ST:, :] = r["vgen"].transpose(2, 0, 1, 3)
        x_out[:, h0:h0 + HPC, 0, :] = r["xout"].transpose(2, 0, 1)
    if _profile:
        return (k_out, v_out, x_out), res
    return (k_out, v_out, x_out)
